# revision 1
# baseline (speedup 1.0000x reference)
"""MiniGPT forward pass on 8 Trainium2 NeuronCores.

Sharding: sequence-parallel. Core c handles batch g = c//4, token chunk
r = c%4 (512 tokens). All per-token ops (LN, QKV, Wo, FFN, LM head) are
local; K/V are exchanged with two 4-core AllGathers per layer (split by
head halves so the second overlaps first-half attention).

Activations are kept transposed [D, tokens] so every major matmul has
free dim N=512 with contraction on the partition axis. The K/Q/V/
attention-weights path runs in bf16 (fp32 PSUM accumulation); LN,
projections, FFN and LM head run in fp32r. Softmax skips
max-subtraction (pre-softmax scores are O(1) here); masked positions
are zeroed exactly by multiplying exp(s) with a 0/1 mask, and the
denominator comes from a ones-column appended to V inside the same
accumulation matmul.
"""
import sys
sys.path.insert(0, '/opt/trn_rl_repo')
import numpy as np
import concourse.bass as bass
import concourse.bacc as bacc
import concourse.tile as tile
import concourse.mybir as mybir
from concourse.bass_utils import run_bass_kernel_spmd

F32 = mybir.dt.float32
F32R = mybir.dt.float32r
BF16 = mybir.dt.bfloat16
AF = mybir.ActivationFunctionType
ALU = mybir.AluOpType

# model dims
B, S, D, H, DK, F, V = 2, 2048, 1024, 16, 64, 4096, 32000
L = 6
EPS = 1e-5
T = 512           # tokens per core
NT = T // 128     # 4 token tiles
ND = D // 128     # 8 d tiles
NF = F // 128     # 32 f tiles
NK = S // 128     # 16 key tiles
NV = V // 128     # 250 vocab tiles
HPAIRS = H // 2   # 8 head-pair tiles
N_CORES = 8

# one AG half: K rows [h*512:(h+1)*512] (4 hp tiles) + V cols [h*520:(h+1)*520]
KTH_FLAT = 4 * 128 * T          # 262144
VH_FLAT = T * 520               # 266240
KVH_FLAT = KTH_FLAT + VH_FLAT   # 528384
VW = H * 65                     # 1040

_CACHE = {}


def _build(n_layers=L, n_vtiles=NV):
    nc = bacc.Bacc("TRN2", target_bir_lowering=False, debug=False,
                   num_devices=N_CORES)

    def din(name, shape, dt=F32R):
        return nc.dram_tensor(name, shape, dt, kind="ExternalInput").ap()

    x0t = din("x0t", [D, T])
    maskt = din("maskt", [S, T], BF16)
    wq = din("wq", [L, D, D])
    wk = din("wk", [L, D, D])
    wv = din("wv", [L, D, D])
    wo = din("wo", [L, D, D])
    w1s = din("w1s", [L, NF, 128, D])
    w2 = din("w2", [L, F, D])
    woutr = din("woutr", [NV, 128, D])
    ln1g = din("ln1g", [L, D], F32)
    ln1b = din("ln1b", [L, D], F32)
    ln2g = din("ln2g", [L, D], F32)
    ln2b = din("ln2b", [L, D], F32)
    b1 = din("b1", [L, F], F32)
    b2 = din("b2", [L, D], F32)
    lnfg = din("lnfg", [D], F32)
    lnfb = din("lnfb", [D], F32)
    ones_in = din("ones_in", [128])
    onescol = din("onescol", [128, H], BF16)

    logits_t = nc.dram_tensor("logits_t", [n_vtiles * 128, T], F32R,
                              kind="ExternalOutput").ap()

    groups = [[0, 1, 2, 3], [4, 5, 6, 7]]

    with tile.TileContext(nc) as tc:
        with tc.tile_pool(name="sb", bufs=1) as sb, \
             tc.tile_pool(name="ps", bufs=1, space="PSUM") as ps, \
             tc.tile_pool(name="dram", bufs=1, space="DRAM") as dram:

            ones128 = sb.tile([128, 1], F32R, tag="ones128")
            ones1 = sb.tile([1, 128], F32R, tag="ones1")
            nc.sync.dma_start(ones128[:], ones_in[:, None])
            nc.sync.dma_start(ones1[:], ones_in[None, :])

            # persistent residual stream xT: 8 tiles [128, 512]
            xts = []
            for j in range(ND):
                t = sb.tile([128, T], F32R, tag="xt", bufs=ND, name=f"xt{j}")
                nc.sync.dma_start(t[:], x0t[j * 128:(j + 1) * 128, :])
                xts.append(t)

            # causal mask, resident whole kernel
            mts = []
            for k in range(NK):
                t = sb.tile([128, T], BF16, tag="mask", bufs=NK, name=f"mask{k}")
                nc.sync.dma_start(t[:], maskt[k * 128:(k + 1) * 128, :])
                mts.append(t)

            def layer_norm(x_tiles, gcol_t, bcol_t, sfx):
                statx = ps.tile([1, T], F32, tag="stat", bufs=2,
                                name=f"stx{sfx}")
                statq = ps.tile([1, T], F32, tag="stat", bufs=2,
                                name=f"stq{sfx}")
                for j in range(ND):
                    sq = sb.tile([128, T], F32R, tag="work512", bufs=4,
                                 name=f"sq{sfx}")
                    nc.scalar.activation(sq[:], x_tiles[j][:], AF.Square)
                    nc.tensor.matmul(statx[0:1, :], ones128[:], x_tiles[j][:],
                                     start=(j == 0), stop=(j == ND - 1))
                    nc.tensor.matmul(statq[0:1, :], ones128[:], sq[:],
                                     start=(j == 0), stop=(j == ND - 1))
                mean = sb.tile([1, T], F32R, tag="lnsmall", bufs=3,
                               name=f"mean{sfx}")
                nc.vector.tensor_scalar_mul(mean[:], statx[0:1, :], 1.0 / D)
                ex2 = sb.tile([1, T], F32, tag="lnsmall", bufs=3,
                              name=f"ex2{sfx}")
                nc.vector.tensor_scalar_mul(ex2[:], statq[0:1, :], 1.0 / D)
                m2 = sb.tile([1, T], F32, tag="lnsmall", bufs=3,
                             name=f"m2{sfx}")
                nc.scalar.activation(m2[:], mean[:], AF.Square)
                var = sb.tile([1, T], F32, tag="lnsmall", bufs=3,
                              name=f"var{sfx}")
                nc.vector.tensor_sub(var[:], ex2[:], m2[:])
                nc.vector.tensor_scalar_add(var[:], var[:], EPS)
                sd = sb.tile([1, T], F32, tag="lnsmall", bufs=3,
                             name=f"sd{sfx}")
                nc.scalar.activation(sd[:], var[:], AF.Sqrt)
                rstd = sb.tile([1, T], F32R, tag="lnsmall", bufs=3,
                               name=f"rstd{sfx}")
                with nc.allow_low_precision(reason="fp32r matmul feed"):
                    nc.vector.reciprocal(rstd[:], sd[:])
                mb = ps.tile([128, T], F32, tag="sT", bufs=2, name=f"mb{sfx}")
                nc.tensor.matmul(mb[:], ones1[:], mean[:], start=True,
                                 stop=True)
                rb = ps.tile([128, T], F32, tag="sT", bufs=2, name=f"rb{sfx}")
                nc.tensor.matmul(rb[:], ones1[:], rstd[:], start=True,
                                 stop=True)
                h_tiles = []
                for j in range(ND):
                    ht = sb.tile([128, T], F32R, tag="hx", bufs=ND,
                                 name=f"h{sfx}_{j}")
                    nc.vector.tensor_sub(ht[:], x_tiles[j][:], mb[:])
                    nc.vector.tensor_mul(ht[:], ht[:], rb[:])
                    nc.vector.tensor_scalar(ht[:], ht[:], gcol_t[:, j:j + 1],
                                            bcol_t[:, j:j + 1], ALU.mult,
                                            ALU.add)
                    h_tiles.append(ht)
                return h_tiles

            for l in range(n_layers):
                # --- per-layer params ---
                lg1 = sb.tile([128, ND], F32, tag="lnp", bufs=8, name="lg1")
                nc.sync.dma_start(lg1[:], ln1g[l].rearrange("(c p) -> p c", p=128))
                lb1 = sb.tile([128, ND], F32, tag="lnp", bufs=8, name="lb1")
                nc.sync.dma_start(lb1[:], ln1b[l].rearrange("(c p) -> p c", p=128))
                lg2 = sb.tile([128, ND], F32, tag="lnp", bufs=8, name="lg2")
                nc.sync.dma_start(lg2[:], ln2g[l].rearrange("(c p) -> p c", p=128))
                lb2 = sb.tile([128, ND], F32, tag="lnp", bufs=8, name="lb2")
                nc.sync.dma_start(lb2[:], ln2b[l].rearrange("(c p) -> p c", p=128))
                b1t = sb.tile([128, NF], F32, tag="b1t", bufs=2, name="b1t")
                nc.sync.dma_start(b1t[:], b1[l].rearrange("(c p) -> p c", p=128))
                b2t = sb.tile([128, ND], F32, tag="lnp", bufs=8, name="b2t")
                nc.sync.dma_start(b2t[:], b2[l].rearrange("(c p) -> p c", p=128))

                h1 = layer_norm(xts, lg1, lb1, f"a{l}")

                kv_in = [dram.tile([KVH_FLAT], BF16, tag=f"kvin{h}", bufs=2,
                                   name=f"kvin{h}") for h in range(2)]

                # --- K projection (all 8 hp tiles), DMA into both halves ---
                wk_t = []
                for ci in range(ND):
                    t = sb.tile([128, D], F32R, tag="w", bufs=9, name=f"wk{ci}")
                    nc.sync.dma_start(t[:], wk[l][ci * 128:(ci + 1) * 128, :])
                    wk_t.append(t)
                for j in range(ND):
                    mm = ps.tile([128, T], F32, tag="mm", bufs=2, name="kmm")
                    for ci in range(ND):
                        nc.tensor.matmul(mm[:], wk_t[ci][:, j * 128:(j + 1) * 128],
                                         h1[ci][:], start=(ci == 0),
                                         stop=(ci == ND - 1))
                    kt = sb.tile([128, T], BF16, tag="ktl", bufs=3, name="ktl")
                    nc.vector.tensor_copy(kt[:], mm[:])
                    half, hp_in = j // 4, j % 4
                    nc.sync.dma_start(
                        kv_in[half][hp_in * 128 * T:(hp_in + 1) * 128 * T]
                        .rearrange("(p n) -> p n", p=128), kt[:])

                # --- V projection; nh half -> AG half ---
                wv_t = []
                for ci in range(ND):
                    t = sb.tile([128, D], F32R, tag="w", bufs=9, name=f"wv{ci}")
                    nc.sync.dma_start(t[:], wv[l][ci * 128:(ci + 1) * 128, :])
                    wv_t.append(t)
                vaug = []
                for tt in range(NT):
                    va = sb.tile([128, VW], BF16, tag="kvg", bufs=16,
                                 name=f"va{tt}")
                    nc.sync.dma_start(
                        va[:, 0:VW].rearrange("p (h c) -> p h c", c=65)[:, :, 64:65],
                        onescol[:, :, None])
                    vaug.append(va)
                for nh in range(2):
                    for tt in range(NT):
                        mm = ps.tile([128, T], F32, tag="mm", bufs=2, name="vmm")
                        for ci in range(ND):
                            nc.tensor.matmul(
                                mm[:],
                                h1[ci][:, tt * 128:(tt + 1) * 128],
                                wv_t[ci][:, nh * 512:(nh + 1) * 512],
                                start=(ci == 0), stop=(ci == ND - 1))
                        nc.vector.tensor_copy(
                            vaug[tt][:, nh * 520:(nh + 1) * 520]
                            .rearrange("p (h c) -> p h c", c=65)[:, :, 0:64],
                            mm[:].rearrange("p (h c) -> p h c", c=64))
                        nc.sync.dma_start(
                            kv_in[nh][KTH_FLAT + tt * 128 * 520:
                                      KTH_FLAT + (tt + 1) * 128 * 520]
                            .rearrange("(p n) -> p n", p=128),
                            vaug[tt][:, nh * 520:(nh + 1) * 520])

                kv_out = []
                for h in range(2):
                    ko = dram.tile([4 * KVH_FLAT], BF16, tag=f"kvout{h}",
                                   bufs=2, name=f"kvout{h}")
                    nc.gpsimd.collective_compute(
                        "AllGather", ALU.bypass, replica_groups=groups,
                        ins=[kv_in[h].opt()], outs=[ko.opt()])
                    kv_out.append(ko)

                # --- Q projection ---
                wq_t = []
                for ci in range(ND):
                    t = sb.tile([128, D], F32R, tag="w", bufs=9, name=f"wq{ci}")
                    nc.sync.dma_start(t[:], wq[l][ci * 128:(ci + 1) * 128, :])
                    wq_t.append(t)
                qts = []
                for j in range(ND):
                    mm = ps.tile([128, T], F32, tag="mm", bufs=2, name="qmm")
                    for ci in range(ND):
                        nc.tensor.matmul(mm[:], wq_t[ci][:, j * 128:(j + 1) * 128],
                                         h1[ci][:], start=(ci == 0),
                                         stop=(ci == ND - 1))
                    qt = sb.tile([128, T], BF16, tag="qt", bufs=ND, name=f"qt{j}")
                    nc.vector.tensor_copy(qt[:], mm[:])
                    qts.append(qt)

                # --- attention (hp 0-3 from AG half 0, hp 4-7 from half 1) ---
                ctx_sb = []
                vfs = None
                for hp in range(HPAIRS):
                    half, hp_in = hp // 4, hp % 4
                    ko = kv_out[half]
                    ktf = sb.tile([128, S], BF16, tag="ktf", bufs=2,
                                  name=f"ktf{hp}")
                    for r in range(4):
                        off = r * KVH_FLAT + hp_in * 128 * T
                        nc.sync.dma_start(
                            ktf[:, r * T:(r + 1) * T],
                            ko[off:off + 128 * T]
                            .rearrange("(p n) -> p n", p=128))
                    if hp_in == 0:
                        # load this half's V tiles [128, 520] x 16
                        vfs = []
                        for kt_i in range(NK):
                            r, tt = kt_i // NT, kt_i % NT
                            off = r * KVH_FLAT + KTH_FLAT + tt * 128 * 520
                            vt = sb.tile([128, 520], BF16, tag="kvg", bufs=16,
                                         name=f"vf{half}_{kt_i}")
                            nc.sync.dma_start(
                                vt[:], ko[off:off + 128 * 520]
                                .rearrange("(p n) -> p n", p=128))
                            vfs.append(vt)
                    cs = sb.tile([128, T], F32R, tag="hx", bufs=ND,
                                 name=f"cs{hp}")
                    ctx_sb.append(cs)
                    ctxp = [ps.tile([65, T], F32, tag="ctxp", bufs=2,
                                    name=f"ctxp{hh}") for hh in range(2)]
                    for kt_i in range(NK):
                        sTs = []
                        for hh in range(2):
                            offp = hh * 64
                            sT = ps.tile([128, T], F32, tag="sT", bufs=2,
                                         name="sT")
                            nc.tensor.matmul(
                                sT[:],
                                ktf[offp:offp + 64,
                                    kt_i * 128:(kt_i + 1) * 128],
                                qts[hp][offp:offp + 64, :],
                                start=True, stop=True)
                            sTs.append(sT)
                        for hh in range(2):
                            h_loc = hp_in * 2 + hh   # head index within half
                            es = sb.tile([128, T], BF16, tag="work512", bufs=4,
                                         name="es")
                            nc.scalar.activation(es[:], sTs[hh][:], AF.Exp)
                            nc.vector.tensor_mul(es[:], es[:], mts[kt_i][:])
                            nc.tensor.matmul(
                                ctxp[hh][:],
                                vfs[kt_i][:, h_loc * 65:h_loc * 65 + 65],
                                es[:], start=(kt_i == 0),
                                stop=(kt_i == NK - 1))
                    for hh in range(2):
                        offp = hh * 64
                        rec = sb.tile([1, T], F32R, tag="lnsmall", bufs=3,
                                      name="rec")
                        with nc.allow_low_precision(reason="fp32r matmul feed"):
                            nc.vector.reciprocal(rec[:], ctxp[hh][64:65, :])
                        rbp = ps.tile([64, T], F32, tag="mm", bufs=2,
                                      name="rbp")
                        nc.tensor.matmul(rbp[:], ones1[0:1, 0:64], rec[:],
                                         start=True, stop=True)
                        nc.vector.tensor_copy(cs[offp:offp + 64, :],
                                              ctxp[hh][0:64, :])
                        nc.vector.tensor_mul(cs[offp:offp + 64, :],
                                             cs[offp:offp + 64, :], rbp[:])

                # --- Wo + residual ---
                wo_t = []
                for ci in range(ND):
                    t = sb.tile([128, D], F32R, tag="w", bufs=9, name=f"wo{ci}")
                    nc.sync.dma_start(t[:], wo[l][ci * 128:(ci + 1) * 128, :])
                    wo_t.append(t)
                for j in range(ND):
                    mm = ps.tile([128, T], F32, tag="mm", bufs=2, name="omm")
                    for ci in range(ND):
                        nc.tensor.matmul(mm[:], wo_t[ci][:, j * 128:(j + 1) * 128],
                                         ctx_sb[ci][:], start=(ci == 0),
                                         stop=(ci == ND - 1))
                    nc.vector.tensor_add(xts[j][:], xts[j][:], mm[:])

                h2 = layer_norm(xts, lg2, lb2, f"b{l}")

                # --- FFN: W1 + gelu for all 32 f-tiles ---
                gts = []
                for fi in range(NF):
                    slab = sb.tile([128, D], F32R, tag="w", bufs=9,
                                   name=f"w1s{fi}")
                    nc.sync.dma_start(slab[:], w1s[l, fi])
                    h3 = ps.tile([128, T], F32, tag="ctxp", bufs=2, name="h3")
                    for ci in range(ND):
                        nc.tensor.matmul(h3[:], slab[:, ci * 128:(ci + 1) * 128],
                                         h2[ci][:], start=(ci == 0),
                                         stop=(ci == ND - 1))
                    if fi % 2 == 0:
                        gt = sb.tile([128, 2 * T], F32R, tag="kvg", bufs=16,
                                     name=f"g{fi // 2}")
                        gts.append(gt)
                    nc.scalar.activation(
                        gts[fi // 2][:, (fi % 2) * T:(fi % 2 + 1) * T],
                        h3[:], AF.Gelu, bias=b1t[:, fi:fi + 1])

                # --- FFN: W2 single pass, 8 psum accumulators ---
                accs = []
                for j in range(ND):
                    tagj = ["mm", "mm", "sT", "sT", "ctxp", "ctxp", "stat",
                            "stat"][j]
                    accs.append(ps.tile([128, T], F32, tag=tagj, bufs=2,
                                        name=f"w2acc{j}"))
                for fi in range(NF):
                    slab = sb.tile([128, D], F32R, tag="w", bufs=9,
                                   name=f"w2s{fi}")
                    nc.sync.dma_start(slab[:], w2[l][fi * 128:(fi + 1) * 128, :])
                    for j in range(ND):
                        nc.tensor.matmul(
                            accs[j][:], slab[:, j * 128:(j + 1) * 128],
                            gts[fi // 2][:, (fi % 2) * T:(fi % 2 + 1) * T],
                            start=(fi == 0), stop=(fi == NF - 1))
                for j in range(ND):
                    nc.vector.scalar_tensor_tensor(
                        xts[j][:], accs[j][:], b2t[:, j:j + 1], xts[j][:],
                        ALU.add, ALU.add)

            # --- final LN ---
            lgf = sb.tile([128, ND], F32, tag="lnp", bufs=8, name="lgf")
            nc.sync.dma_start(lgf[:], lnfg.rearrange("(c p) -> p c", p=128))
            lbf = sb.tile([128, ND], F32, tag="lnp", bufs=8, name="lbf")
            nc.sync.dma_start(lbf[:], lnfb.rearrange("(c p) -> p c", p=128))
            hf = layer_norm(xts, lgf, lbf, "f")

            # --- LM head: vocab tiles ---
            for v in range(n_vtiles):
                slab = sb.tile([128, D], F32R, tag="w", bufs=9,
                               name=f"wouts{v}")
                nc.sync.dma_start(slab[:], woutr[v])
                mm = ps.tile([128, T], F32, tag="mm", bufs=2, name="lmm")
                for ci in range(ND):
                    nc.tensor.matmul(mm[:], slab[:, ci * 128:(ci + 1) * 128],
                                     hf[ci][:], start=(ci == 0),
                                     stop=(ci == ND - 1))
                ot = sb.tile([128, T], F32R, tag="work512", bufs=4, name="ot")
                nc.vector.tensor_copy(ot[:], mm[:])
                nc.sync.dma_start(logits_t[v * 128:(v + 1) * 128, :], ot[:])

    nc.compile()
    return nc


def get_program(n_layers=L, n_vtiles=NV):
    key = (n_layers, n_vtiles)
    if key not in _CACHE:
        _CACHE[key] = _build(n_layers, n_vtiles)
    return _CACHE[key]


def prep_inputs(tokens, tok_emb, pos_emb, Wq, Wk, Wv, Wo, ln1_g, ln1_b,
                ln2_g, ln2_b, W1, b1, W2, b2, lnf_g, lnf_b, Wout):
    import ml_dtypes
    tokens = np.asarray(tokens)
    f = lambda a: np.ascontiguousarray(np.asarray(a, dtype=np.float32))
    tok_emb, pos_emb = f(tok_emb), f(pos_emb)
    Wq, Wk, Wv, Wo = f(Wq), f(Wk), f(Wv), f(Wo)
    W1, W2, Wout = f(W1), f(W2), f(Wout)
    ln1_g, ln1_b, ln2_g, ln2_b = f(ln1_g), f(ln1_b), f(ln2_g), f(ln2_b)
    b1a, b2a, lnf_g, lnf_b = f(b1), f(b2), f(lnf_g), f(lnf_b)

    wq_s = np.ascontiguousarray(Wq / np.sqrt(DK))   # fold 1/sqrt(dk) into Q
    w1s = np.ascontiguousarray(
        W1.reshape(L, ND, 128, NF, 128).transpose(0, 3, 2, 1, 4)
        .reshape(L, NF, 128, D))
    woutr = np.ascontiguousarray(
        Wout.reshape(ND, 128, NV, 128).transpose(2, 1, 0, 3)
        .reshape(NV, 128, D))
    ones_in = np.ones(128, np.float32)
    onescol = np.ones((128, H), ml_dtypes.bfloat16)

    shared = dict(wq=wq_s, wk=Wk, wv=Wv, wo=Wo, w1s=w1s, w2=W2, woutr=woutr,
                  ln1g=ln1_g, ln1b=ln1_b, ln2g=ln2_g, ln2b=ln2_b,
                  b1=b1a, b2=b2a, lnfg=lnf_g, lnfb=lnf_b,
                  ones_in=ones_in, onescol=onescol)

    in_maps = []
    for c in range(N_CORES):
        g, r = c // 4, c % 4
        toks = tokens[g, r * T:(r + 1) * T]
        x0 = tok_emb[toks] + pos_emb[r * T:(r + 1) * T]
        x0t = np.ascontiguousarray(x0.T)
        k_idx = np.arange(S)[:, None]
        q_idx = r * T + np.arange(T)[None, :]
        maskt = (k_idx <= q_idx).astype(ml_dtypes.bfloat16)
        m = dict(shared)
        m["x0t"] = x0t
        m["maskt"] = maskt
        in_maps.append(m)
    return in_maps


def kernel(**inputs):
    nc = get_program()
    in_maps = prep_inputs(**inputs)
    res = run_bass_kernel_spmd(nc, in_maps, list(range(N_CORES)))
    out = np.empty((B, S, V), np.float32)
    for c in range(N_CORES):
        g, r = c // 4, c % 4
        out[g, r * T:(r + 1) * T, :] = res.results[c]["logits_t"].T
    return out



# revision 6
# speedup vs baseline: 1.3218x; 1.3218x over previous
"""MiniGPT forward pass on 8 Trainium2 NeuronCores.

Sharding: block-interleaved sequence parallel. Core c handles batch
g = c//4; within the batch, rank r = c%4 owns token blocks
{128*(4m+r) : m=0..3} (512 tokens). The interleaving makes the causal
tile structure identical on every core (SPMD), so attention skips the
~40% of key tiles that are entirely in the future for every query
block: query block qt (128 queries) only attends key tiles kt with
kt <= 4*qt+3. Per-core masks (data, not code) handle the per-rank
diagonal offset.

K/V are exchanged with two 4-core AllGathers per layer (split by head
halves; AG0 launches right after half-0 K/V so half-1 projection + Q
overlap it). All weights live in bf16 (half the HBM traffic of fp32);
activations feeding matmuls are bf16; LayerNorm stats run in fp32 via
PE ones-matmuls, with rstd = exp(-0.5*ln(var+eps)) so the scalar
engine never needs the sqrt table (exp/ln/square share one table set;
only gelu forces a table switch). Softmax skips max-subtraction;
masked positions are zeroed by multiplying exp(s) with a 0/1 mask on
the diagonal tile groups only, and the denominator comes from a
ones-column appended to V inside the same accumulation matmul.
"""
import sys
sys.path.insert(0, '/opt/trn_rl_repo')
import numpy as np
import concourse.bass as bass
import concourse.bacc as bacc
import concourse.tile as tile
import concourse.mybir as mybir
from concourse.bass_utils import run_bass_kernel_spmd

F32 = mybir.dt.float32
F32R = mybir.dt.float32r
BF16 = mybir.dt.bfloat16
AF = mybir.ActivationFunctionType
ALU = mybir.AluOpType

# model dims
B, S, D, H, DK, F, V = 2, 2048, 1024, 16, 64, 4096, 32000
L = 6
EPS = 1e-5
T = 512           # tokens per core
NT = T // 128     # 4 token blocks per core
ND = D // 128     # 8 d tiles
NF = F // 128     # 32 f tiles
NK = S // 128     # 16 key tiles
NV = V // 128     # 250 vocab tiles
HPAIRS = H // 2   # 8 head-pair tiles
N_CORES = 8

# one AG half: K rows [h*512:(h+1)*512] (4 hp tiles) + V cols [h*520:(h+1)*520]
KTH_FLAT = 4 * 128 * T          # 262144
VH_FLAT = T * 520               # 266240
KVH_FLAT = KTH_FLAT + VH_FLAT   # 528384
VW = H * 65                     # 1040

_CACHE = {}


def _build(n_layers=L, n_vtiles=NV):
    nc = bacc.Bacc("TRN2", target_bir_lowering=False, debug=False,
                   num_devices=N_CORES)

    def din(name, shape, dt=BF16):
        return nc.dram_tensor(name, shape, dt, kind="ExternalInput").ap()

    x0t = din("x0t", [D, T], F32R)
    maskt = din("maskt", [128, 4 * 128])       # [key, rank, query] diag mask
    wq = din("wq", [L, D, D])
    wk = din("wk", [L, D, D])
    wv = din("wv", [L, D, D])
    wo = din("wo", [L, D, D])
    w1s = din("w1s", [L, NF, 128, D])
    w2 = din("w2", [L, F, D])
    woutr = din("woutr", [NV, 128, D])
    ln1g = din("ln1g", [L, D], F32)
    ln1b = din("ln1b", [L, D], F32)
    ln2g = din("ln2g", [L, D], F32)
    ln2b = din("ln2b", [L, D], F32)
    b1 = din("b1", [L, F], F32)
    b2 = din("b2", [L, D], F32)
    lnfg = din("lnfg", [D], F32)
    lnfb = din("lnfb", [D], F32)
    ones_in = din("ones_in", [128], F32R)
    onescol = din("onescol", [128, H], BF16)

    logits_t = nc.dram_tensor("logits_t", [n_vtiles * 128, T], BF16,
                              kind="ExternalOutput").ap()

    groups = [[0, 1, 2, 3], [4, 5, 6, 7]]

    with tile.TileContext(nc) as tc:
        with tc.tile_pool(name="sb", bufs=1) as sb, \
             tc.tile_pool(name="ps", bufs=1, space="PSUM") as ps, \
             tc.tile_pool(name="dram", bufs=1, space="DRAM") as dram:

            ones128 = sb.tile([128, 1], F32R, tag="ones128")
            ones1 = sb.tile([1, 128], F32R, tag="ones1")
            nc.sync.dma_start(ones128[:], ones_in[:, None])
            nc.sync.dma_start(ones1[:], ones_in[None, :])

            # persistent residual stream xT: 8 tiles [128, 512] fp32
            xts = []
            for j in range(ND):
                t = sb.tile([128, T], F32R, tag="xt", bufs=ND, name=f"xt{j}")
                nc.sync.dma_start(t[:], x0t[j * 128:(j + 1) * 128, :])
                xts.append(t)

            # diagonal causal mask [key, rank, query], resident whole kernel
            mt = sb.tile([128, 4 * 128], BF16, tag="mask", name="mask")
            nc.sync.dma_start(mt[:], maskt[:])
            mt3 = mt.rearrange("p (c q) -> p c q", q=128)

            def layer_norm(x_tiles, gcol_t, bcol_t, sfx):
                # stats in one psum bank-pair: sum(x) cols 0:T, sum(x^2) T:2T
                stat = ps.tile([1, 2 * T], F32, tag="big", bufs=3,
                               name=f"st{sfx}")
                for j in range(ND):
                    sq = sb.tile([128, T], F32R, tag="work512", bufs=4,
                                 name=f"sq{sfx}")
                    nc.scalar.activation(sq[:], x_tiles[j][:], AF.Square)
                    nc.tensor.matmul(stat[0:1, 0:T], ones128[:],
                                     x_tiles[j][:],
                                     start=(j == 0), stop=(j == ND - 1))
                    nc.tensor.matmul(stat[0:1, T:2 * T], ones128[:], sq[:],
                                     start=(j == 0), stop=(j == ND - 1))
                mean = sb.tile([1, T], F32R, tag="lnsmall", bufs=3,
                               name=f"mean{sfx}")
                nc.vector.tensor_scalar_mul(mean[:], stat[0:1, 0:T], 1.0 / D)
                ex2 = sb.tile([1, T], F32, tag="lnsmall", bufs=3,
                              name=f"ex2{sfx}")
                nc.vector.tensor_scalar_mul(ex2[:], stat[0:1, T:2 * T],
                                            1.0 / D)
                m2 = sb.tile([1, T], F32, tag="lnsmall", bufs=3,
                             name=f"m2{sfx}")
                nc.scalar.activation(m2[:], mean[:], AF.Square)
                var = sb.tile([1, T], F32, tag="lnsmall", bufs=3,
                              name=f"var{sfx}")
                nc.vector.tensor_sub(var[:], ex2[:], m2[:])
                nc.vector.tensor_scalar_add(var[:], var[:], EPS)
                # rstd = exp(-0.5 * ln(var)) — keeps ACT on the exp table set
                lv = sb.tile([1, T], F32, tag="lnsmall", bufs=3,
                             name=f"lv{sfx}")
                nc.scalar.activation(lv[:], var[:], AF.Ln)
                rstd = sb.tile([1, T], F32R, tag="lnsmall", bufs=3,
                               name=f"rstd{sfx}")
                with nc.allow_low_precision(reason="fp32r matmul feed"):
                    nc.scalar.activation(rstd[:], lv[:], AF.Exp, scale=-0.5)
                mb = ps.tile([128, T], F32, tag="ctxp", bufs=2,
                             name=f"mb{sfx}")
                nc.tensor.matmul(mb[:], ones1[:], mean[:], start=True,
                                 stop=True)
                rb = ps.tile([128, T], F32, tag="ctxp", bufs=2,
                             name=f"rb{sfx}")
                nc.tensor.matmul(rb[:], ones1[:], rstd[:], start=True,
                                 stop=True)
                h_tiles = []
                for j in range(ND):
                    ht = sb.tile([128, T], BF16, tag="hx", bufs=ND,
                                 name=f"h{sfx}_{j}")
                    hw = sb.tile([128, T], F32, tag="work512", bufs=4,
                                 name=f"hw{sfx}")
                    nc.vector.tensor_sub(hw[:], x_tiles[j][:], mb[:])
                    nc.vector.tensor_mul(hw[:], hw[:], rb[:])
                    with nc.allow_low_precision(reason="bf16 matmul feed"):
                        nc.vector.tensor_scalar(ht[:], hw[:],
                                                gcol_t[:, j:j + 1],
                                                bcol_t[:, j:j + 1], ALU.mult,
                                                ALU.add)
                    h_tiles.append(ht)
                return h_tiles

            for l in range(n_layers):
                # --- per-layer params ---
                lg1 = sb.tile([128, ND], F32, tag="lnp", bufs=8, name="lg1")
                nc.sync.dma_start(lg1[:], ln1g[l].rearrange("(c p) -> p c", p=128))
                lb1 = sb.tile([128, ND], F32, tag="lnp", bufs=8, name="lb1")
                nc.sync.dma_start(lb1[:], ln1b[l].rearrange("(c p) -> p c", p=128))
                lg2 = sb.tile([128, ND], F32, tag="lnp", bufs=8, name="lg2")
                nc.sync.dma_start(lg2[:], ln2g[l].rearrange("(c p) -> p c", p=128))
                lb2 = sb.tile([128, ND], F32, tag="lnp", bufs=8, name="lb2")
                nc.sync.dma_start(lb2[:], ln2b[l].rearrange("(c p) -> p c", p=128))
                b1t = sb.tile([128, NF], F32, tag="b1t", bufs=2, name="b1t")
                nc.sync.dma_start(b1t[:], b1[l].rearrange("(c p) -> p c", p=128))
                b2t = sb.tile([128, ND], F32, tag="lnp", bufs=8, name="b2t")
                nc.sync.dma_start(b2t[:], b2[l].rearrange("(c p) -> p c", p=128))

                h1 = layer_norm(xts, lg1, lb1, f"a{l}")

                kv_in = [dram.tile([KVH_FLAT], BF16, tag=f"kvin{h}", bufs=2,
                                   name=f"kvin{h}") for h in range(2)]

                wk_t = []
                for ci in range(ND):
                    t = sb.tile([128, D], BF16, tag="w", bufs=18, name=f"wk{ci}")
                    nc.sync.dma_start(t[:], wk[l][ci * 128:(ci + 1) * 128, :])
                    wk_t.append(t)
                wv_t = []
                for ci in range(ND):
                    t = sb.tile([128, D], BF16, tag="w", bufs=18, name=f"wv{ci}")
                    nc.sync.dma_start(t[:], wv[l][ci * 128:(ci + 1) * 128, :])
                    wv_t.append(t)
                vaug = []
                for tt in range(NT):
                    va = sb.tile([128, VW], BF16, tag="kvg", bufs=16,
                                 name=f"va{tt}")
                    nc.sync.dma_start(
                        va[:, 0:VW].rearrange("p (h c) -> p h c", c=65)[:, :, 64:65],
                        onescol[:, :, None])
                    vaug.append(va)

                kv_out = []
                for nh in range(2):
                    # K projection for this half's 4 hp tiles
                    for j in range(nh * 4, nh * 4 + 4):
                        mm = ps.tile([128, 2 * T], F32, tag="big", bufs=3,
                                     name="kmm")
                        for ci in range(ND):
                            nc.tensor.matmul(
                                mm[:, 0:T],
                                wk_t[ci][:, j * 128:(j + 1) * 128],
                                h1[ci][:], start=(ci == 0),
                                stop=(ci == ND - 1))
                        kt = sb.tile([128, T], BF16, tag="ktl", bufs=3,
                                     name="ktl")
                        nc.vector.tensor_copy(kt[:], mm[:, 0:T])
                        hp_in = j % 4
                        nc.sync.dma_start(
                            kv_in[nh][hp_in * 128 * T:(hp_in + 1) * 128 * T]
                            .rearrange("(p n) -> p n", p=128), kt[:])
                    # V projection for this half
                    for tt in range(NT):
                        mm = ps.tile([128, 2 * T], F32, tag="big", bufs=3,
                                     name="vmm")
                        for ci in range(ND):
                            nc.tensor.matmul(
                                mm[:, 0:T],
                                h1[ci][:, tt * 128:(tt + 1) * 128],
                                wv_t[ci][:, nh * 512:(nh + 1) * 512],
                                start=(ci == 0), stop=(ci == ND - 1))
                        nc.vector.tensor_copy(
                            vaug[tt][:, nh * 520:(nh + 1) * 520]
                            .rearrange("p (h c) -> p h c", c=65)[:, :, 0:64],
                            mm[:, 0:T].rearrange("p (h c) -> p h c", c=64))
                        nc.sync.dma_start(
                            kv_in[nh][KTH_FLAT + tt * 128 * 520:
                                      KTH_FLAT + (tt + 1) * 128 * 520]
                            .rearrange("(p n) -> p n", p=128),
                            vaug[tt][:, nh * 520:(nh + 1) * 520])
                    ko = dram.tile([4 * KVH_FLAT], BF16, tag=f"kvout{nh}",
                                   bufs=2, name=f"kvout{nh}")
                    nc.gpsimd.collective_compute(
                        "AllGather", ALU.bypass, replica_groups=groups,
                        ins=[kv_in[nh].opt()], outs=[ko.opt()])
                    kv_out.append(ko)

                # --- Q projection ---
                wq_t = []
                for ci in range(ND):
                    t = sb.tile([128, D], BF16, tag="w", bufs=18, name=f"wq{ci}")
                    nc.sync.dma_start(t[:], wq[l][ci * 128:(ci + 1) * 128, :])
                    wq_t.append(t)
                qts = []
                for j in range(ND):
                    mm = ps.tile([128, 2 * T], F32, tag="big", bufs=3,
                                 name="qmm")
                    for ci in range(ND):
                        nc.tensor.matmul(mm[:, 0:T],
                                         wq_t[ci][:, j * 128:(j + 1) * 128],
                                         h1[ci][:], start=(ci == 0),
                                         stop=(ci == ND - 1))
                    qt = sb.tile([128, T], BF16, tag="qt", bufs=ND, name=f"qt{j}")
                    nc.vector.tensor_copy(qt[:], mm[:, 0:T])
                    qts.append(qt)

                # --- attention (hp 0-3 from AG half 0, hp 4-7 from half 1) ---
                # ktf free-dim layout is rank-major: rank c's local keys at
                # cols [c*512, (c+1)*512); key tile (rank c, local block b)
                # holds global positions [128*(4b+c), 128*(4b+c)+128).
                ctx_sb = []
                vfs = None
                for hp in range(HPAIRS):
                    half, hp_in = hp // 4, hp % 4
                    ko = kv_out[half]
                    ktf = sb.tile([128, S], BF16, tag="ktf", bufs=2,
                                  name=f"ktf{hp}")
                    for r in range(4):
                        off = r * KVH_FLAT + hp_in * 128 * T
                        nc.sync.dma_start(
                            ktf[:, r * T:(r + 1) * T],
                            ko[off:off + 128 * T]
                            .rearrange("(p n) -> p n", p=128))
                    if hp_in == 0:
                        # load this half's V tiles [128, 520] x 16 (rank-major
                        # index, but DMA'd block-major so group-0 arrives first)
                        vfs = [None] * NK
                        for tt in range(NT):
                            for r in range(4):
                                kt_i = r * NT + tt
                                off = (r * KVH_FLAT + KTH_FLAT
                                       + tt * 128 * 520)
                                vt = sb.tile([128, 520], BF16, tag="kvg",
                                             bufs=16, name=f"vf{half}_{kt_i}")
                                nc.sync.dma_start(
                                    vt[:], ko[off:off + 128 * 520]
                                    .rearrange("(p n) -> p n", p=128))
                                vfs[kt_i] = vt
                    cs = sb.tile([128, T], BF16, tag="hx", bufs=ND,
                                 name=f"cs{hp}")
                    ctx_sb.append(cs)
                    ctxp = [ps.tile([128, T], F32, tag="ctxp", bufs=2,
                                    name=f"ctxp{hh}") for hh in range(2)]
                    # causal groups: key block-group g is needed by query
                    # blocks qt >= g, i.e. query cols [128g, 512).
                    # Software-pipelined: ctx matmuls lag LAG groups behind
                    # the score matmuls so the in-order PE queue never waits
                    # on the exp->mask chain.
                    items = [(g, p, hh) for g in range(NT)
                             for p in range(2) for hh in range(2)]
                    LAG = 2
                    pend = []

                    def emit_ctx(g, p, hh, es3):
                        h_loc = hp_in * 2 + hh
                        ng = T - 128 * g
                        for ci in range(2):
                            c = 2 * p + ci
                            kt_i = c * NT + g
                            nc.tensor.matmul(
                                ctxp[hh][0:65, 128 * g:T],
                                vfs[kt_i][:, h_loc * 65:h_loc * 65 + 65],
                                es3[:, ci, 0:ng],
                                start=(g == 0 and p == 0 and ci == 0),
                                stop=(g == NT - 1 and p == 1 and ci == 1))

                    for it, (g, p, hh) in enumerate(items):
                        ng = T - 128 * g
                        sg = ps.tile([128, 2 * T], F32, tag="big",
                                     bufs=3, name="sg")
                        for ci in range(2):
                            c = 2 * p + ci
                            nc.tensor.matmul(
                                sg[:, ci * T:ci * T + ng],
                                ktf[hh * 64:hh * 64 + 64,
                                    c * T + 128 * g:c * T + 128 * g + 128],
                                qts[hp][hh * 64:hh * 64 + 64, 128 * g:T],
                                start=True, stop=True)
                        es = sb.tile([128, 2 * T], BF16, tag="es",
                                     bufs=6, name="es")
                        es3 = es.rearrange("p (i n) -> p i n", i=2)
                        nc.scalar.activation(
                            es3[:, :, 0:ng],
                            sg.rearrange("p (i n) -> p i n", i=2)[:, :, 0:ng],
                            AF.Exp)
                        # mask only the diagonal query block (qt == g),
                        # the first 128 cols of each sub-tile
                        nc.vector.tensor_mul(
                            es3[:, :, 0:128], es3[:, :, 0:128],
                            mt3[:, 2 * p:2 * p + 2, :])
                        pend.append(((g, p, hh), es3))
                        if len(pend) > LAG:
                            (gg, pp, hh2), e3 = pend.pop(0)
                            emit_ctx(gg, pp, hh2, e3)
                    for (gg, pp, hh2), e3 in pend:
                        emit_ctx(gg, pp, hh2, e3)
                    for hh in range(2):
                        offp = hh * 64
                        rec = sb.tile([1, T], F32R, tag="lnsmall", bufs=3,
                                      name="rec")
                        with nc.allow_low_precision(reason="fp32r matmul feed"):
                            nc.vector.reciprocal(rec[:], ctxp[hh][64:65, :])
                        rbp = ps.tile([128, 2 * T], F32, tag="big", bufs=3,
                                      name="rbp")
                        nc.tensor.matmul(rbp[0:64, 0:T], ones1[0:1, 0:64],
                                         rec[:], start=True, stop=True)
                        cw = sb.tile([64, T], F32, tag="cw", bufs=4,
                                     name="cw")
                        nc.vector.tensor_copy(cw[:], ctxp[hh][0:64, :])
                        with nc.allow_low_precision(reason="bf16 matmul feed"):
                            nc.vector.tensor_mul(cs[offp:offp + 64, :],
                                                 cw[:], rbp[0:64, 0:T])

                # --- Wo + residual ---
                wo_t = []
                for ci in range(ND):
                    t = sb.tile([128, D], BF16, tag="w", bufs=18, name=f"wo{ci}")
                    nc.sync.dma_start(t[:], wo[l][ci * 128:(ci + 1) * 128, :])
                    wo_t.append(t)
                for j in range(ND):
                    mm = ps.tile([128, 2 * T], F32, tag="big", bufs=3,
                                 name="omm")
                    for ci in range(ND):
                        nc.tensor.matmul(mm[:, 0:T],
                                         wo_t[ci][:, j * 128:(j + 1) * 128],
                                         ctx_sb[ci][:], start=(ci == 0),
                                         stop=(ci == ND - 1))
                    nc.vector.tensor_add(xts[j][:], xts[j][:], mm[:, 0:T])

                h2 = layer_norm(xts, lg2, lb2, f"b{l}")

                # --- FFN: W1 + gelu for all 32 f-tiles ---
                gts = []
                for fi in range(NF):
                    slab = sb.tile([128, D], BF16, tag="w", bufs=18,
                                   name=f"w1s{fi}")
                    nc.sync.dma_start(slab[:], w1s[l, fi])
                    h3 = ps.tile([128, 2 * T], F32, tag="big", bufs=3,
                                 name="h3")
                    for ci in range(ND):
                        nc.tensor.matmul(h3[:, 0:T],
                                         slab[:, ci * 128:(ci + 1) * 128],
                                         h2[ci][:], start=(ci == 0),
                                         stop=(ci == ND - 1))
                    if fi % 2 == 0:
                        gt = sb.tile([128, 2 * T], BF16, tag="kvg", bufs=16,
                                     name=f"g{fi // 2}")
                        gts.append(gt)
                    nc.scalar.activation(
                        gts[fi // 2][:, (fi % 2) * T:(fi % 2 + 1) * T],
                        h3[:, 0:T], AF.Gelu, bias=b1t[:, fi:fi + 1])

                # --- FFN: W2 single pass, 8 psum accumulators ---
                accs = []
                for jp in range(3):
                    big = ps.tile([128, 2 * T], F32, tag="big", bufs=3,
                                  name=f"w2b{jp}")
                    accs.append(big[:, 0:T])
                    accs.append(big[:, T:2 * T])
                for jp in range(2):
                    cx = ps.tile([128, T], F32, tag="ctxp", bufs=2,
                                 name=f"w2c{jp}")
                    accs.append(cx[:])
                for fi in range(NF):
                    slab = sb.tile([128, D], BF16, tag="w", bufs=18,
                                   name=f"w2s{fi}")
                    nc.sync.dma_start(slab[:], w2[l][fi * 128:(fi + 1) * 128, :])
                    for j in range(ND):
                        nc.tensor.matmul(
                            accs[j], slab[:, j * 128:(j + 1) * 128],
                            gts[fi // 2][:, (fi % 2) * T:(fi % 2 + 1) * T],
                            start=(fi == 0), stop=(fi == NF - 1))
                for j in range(ND):
                    nc.vector.scalar_tensor_tensor(
                        xts[j][:], accs[j], b2t[:, j:j + 1], xts[j][:],
                        ALU.add, ALU.add)

            # --- final LN ---
            lgf = sb.tile([128, ND], F32, tag="lnp", bufs=8, name="lgf")
            nc.sync.dma_start(lgf[:], lnfg.rearrange("(c p) -> p c", p=128))
            lbf = sb.tile([128, ND], F32, tag="lnp", bufs=8, name="lbf")
            nc.sync.dma_start(lbf[:], lnfb.rearrange("(c p) -> p c", p=128))
            hf = layer_norm(xts, lgf, lbf, "f")

            # --- LM head: vocab tiles ---
            for v in range(n_vtiles):
                slab = sb.tile([128, D], BF16, tag="w", bufs=18,
                               name=f"wouts{v}")
                nc.sync.dma_start(slab[:], woutr[v])
                mm = ps.tile([128, 2 * T], F32, tag="big", bufs=3, name="lmm")
                for ci in range(ND):
                    nc.tensor.matmul(mm[:, 0:T],
                                     slab[:, ci * 128:(ci + 1) * 128],
                                     hf[ci][:], start=(ci == 0),
                                     stop=(ci == ND - 1))
                ot = sb.tile([128, T], BF16, tag="work512b", bufs=4, name="ot")
                nc.vector.tensor_copy(ot[:], mm[:, 0:T])
                nc.sync.dma_start(logits_t[v * 128:(v + 1) * 128, :], ot[:])

    nc.compile()
    return nc


def get_program(n_layers=L, n_vtiles=NV):
    key = (n_layers, n_vtiles)
    if key not in _CACHE:
        _CACHE[key] = _build(n_layers, n_vtiles)
    return _CACHE[key]


def _core_token_idx(r):
    return np.concatenate(
        [np.arange(128 * (4 * m + r), 128 * (4 * m + r) + 128)
         for m in range(NT)])


def prep_inputs(tokens, tok_emb, pos_emb, Wq, Wk, Wv, Wo, ln1_g, ln1_b,
                ln2_g, ln2_b, W1, b1, W2, b2, lnf_g, lnf_b, Wout):
    import ml_dtypes
    BF = ml_dtypes.bfloat16
    tokens = np.asarray(tokens)
    f = lambda a: np.ascontiguousarray(np.asarray(a, dtype=np.float32))
    g16 = lambda a: np.ascontiguousarray(
        np.asarray(a, dtype=np.float32).astype(BF))
    tok_emb, pos_emb = f(tok_emb), f(pos_emb)
    W1a, W2a, Wouta = f(W1), f(W2), f(Wout)
    ln1_g, ln1_b, ln2_g, ln2_b = f(ln1_g), f(ln1_b), f(ln2_g), f(ln2_b)
    b1a, b2a, lnf_g, lnf_b = f(b1), f(b2), f(lnf_g), f(lnf_b)

    wq_s = g16(np.asarray(Wq, np.float32) / np.sqrt(DK))
    w1s = g16(W1a.reshape(L, ND, 128, NF, 128).transpose(0, 3, 2, 1, 4)
              .reshape(L, NF, 128, D))
    woutr = g16(Wouta.reshape(ND, 128, NV, 128).transpose(2, 1, 0, 3)
                .reshape(NV, 128, D))
    ones_in = np.ones(128, np.float32)
    onescol = np.ones((128, H), BF)

    shared = dict(wq=wq_s, wk=g16(Wk), wv=g16(Wv), wo=g16(Wo), w1s=w1s,
                  w2=g16(W2a), woutr=woutr,
                  ln1g=ln1_g, ln1b=ln1_b, ln2g=ln2_g, ln2b=ln2_b,
                  b1=b1a, b2=b2a, lnfg=lnf_g, lnfb=lnf_b,
                  ones_in=ones_in, onescol=onescol)

    in_maps = []
    for core in range(N_CORES):
        g, r = core // 4, core % 4
        idx = _core_token_idx(r)
        toks = tokens[g, idx]
        x0 = tok_emb[toks] + pos_emb[idx]
        x0t = np.ascontiguousarray(x0.T)
        # mask[k, c, j] = key (rank c, in-tile k) visible to query
        # (rank r, in-block j): 128c + k <= 128r + j
        k = np.arange(128)[:, None, None]
        c = np.arange(4)[None, :, None]
        j = np.arange(128)[None, None, :]
        m = (128 * c + k <= 128 * r + j).astype(BF).reshape(128, 512)
        mm = dict(shared)
        mm["x0t"] = x0t
        mm["maskt"] = np.ascontiguousarray(m)
        in_maps.append(mm)
    return in_maps


def kernel(**inputs):
    nc = get_program()
    in_maps = prep_inputs(**inputs)
    res = run_bass_kernel_spmd(nc, in_maps, list(range(N_CORES)))
    out = np.empty((B, S, V), np.float32)
    for core in range(N_CORES):
        g, r = core // 4, core % 4
        idx = _core_token_idx(r)
        out[g, idx, :] = res.results[core]["logits_t"].T.astype(np.float32)
    return out
